# revision 1
# baseline (speedup 1.0000x reference)
"""Multi-head attention (B=2, S=2048, D=1024, H=16, Dh=64) on 8 TRN2 cores.

Sharding: data-parallel over batch (2) x tensor-parallel over heads (16 -> 4
groups of 4). Core c handles batch c//4, heads [4*(c%4), 4*(c%4)+4).
Each core computes its partial output projection (Wo column slice); the host
sums the 4 partials per batch (the "all-reduce") and adds bo.

Device-side per core:
  Q_T/K_T/V_T = W @ X.T via PE, V transposed back to [s, j] via PE transpose.
  Per head: scores_T[k,q] = (K_T-tile).T @ Q_T (K=64 contraction, psum f32),
  exp on ACT (no max subtraction: scores ~ N(0,1), exp never overflows f32),
  attn@V with a ones-column appended to V so row 64 of the PSUM accumulator
  collects the softmax denominator; normalize via DVE reciprocal + a K=1
  ones-matmul partition-broadcast; then the Wo projection in fp32r.

Schedule choices (driven by the cost model; PE executes in emission order):
  - q/k stream + project first so the exp pipeline starts as early as
    possible; Q_T/K_T psum evacuation on ScalarE (idle until the first exp)
  - the V projection/transpose jobs are interleaved INTO the first heads'
    attention loops (xv arrives during attention; the attn@V lag behind exp
    is absorbed by a deep exp-tile pool); V jobs borrow the spare attn@V
    accumulator PSUM slot so scores keep double-buffering
  - normalization of head-task i is emitted after head-task i+1's kb-loop
    so the PE never stalls on the DVE reciprocal
  - attention runs q-half-outer; the first q-half's output projection is
    emitted in slices interleaved into the second q-half's attention

ATT_F16=True streams x/w and runs the attention matmuls in fp16 (~1e-3
rel err, halves the DMA); False keeps everything fp32r (~4e-4 rel err).
Scores, softmax denominators and all PSUM accumulation stay f32 either way.
"""

import numpy as np
from contextlib import ExitStack

import concourse.bass as bass
from concourse import bacc
import concourse.mybir as mybir
import concourse.tile as tile

F32 = mybir.dt.float32
F32R = mybir.dt.float32r
F16 = mybir.dt.float16
AF = mybir.ActivationFunctionType

ATT_F16 = True

B = 2
S = 2048
D = 1024
H = 16
DH = 64
NCORES = 8
HL = 4          # heads per core
J = HL * DH     # 256 local projection width
P = 128
KD = D // P     # 8 d-chunks
NS = S // 512   # 4 s-tiles of 512
KB = S // P     # 16 k-blocks
QH = S // 1024  # 2 q-halves of 1024
EB = D // P     # 8 e-blocks

XDT = F16 if ATT_F16 else F32R        # streamed x / w dtype for q,k,v path
EX_BUFS = 10 if ATT_F16 else 8


def build_nc():
    nc = bacc.Bacc()

    xq = nc.dram_tensor("xq", [P, KD, S], XDT, kind="ExternalInput")
    xk = nc.dram_tensor("xk", [P, KD, S], XDT, kind="ExternalInput")
    xv = nc.dram_tensor("xv", [P, KD, S], XDT, kind="ExternalInput")
    wq = nc.dram_tensor("wq", [P, KD, J], XDT, kind="ExternalInput")
    wk = nc.dram_tensor("wk", [P, KD, J], XDT, kind="ExternalInput")
    wv = nc.dram_tensor("wv", [P, KD, J], XDT, kind="ExternalInput")
    wo = nc.dram_tensor("wo", [P, 2, D], F32R, kind="ExternalInput")
    out_t = nc.dram_tensor("out_t", [EB, P, S], F32, kind="ExternalOutput")

    with tile.TileContext(nc) as tc, ExitStack() as st:
        const = st.enter_context(tc.tile_pool(name="const", bufs=1))
        persist = st.enter_context(tc.tile_pool(name="persist", bufs=1))
        xpool = st.enter_context(tc.tile_pool(name="xstream", bufs=8 if ATT_F16 else 4))

        wq_sb = const.tile([P, KD, J], XDT, tag="wq")
        wk_sb = const.tile([P, KD, J], XDT, tag="wk")
        wv_sb = const.tile([P, KD, J], XDT, tag="wv")
        wo_sb = const.tile([P, 2, D], F32R, tag="wo")

        qt_sb = persist.tile([P, 2, S], XDT, tag="qt")   # Q_T [256, 2048]
        kt_sb = persist.tile([P, 2, S], XDT, tag="kt")   # K_T
        vt_sb = persist.tile([P, 2, S], XDT, tag="vt")   # V_T, pre-transpose
        v_sb = persist.tile([P, KB, HL * (DH + 1)], XDT, tag="v")  # V + ones
        ao_sb = persist.tile([P, 2, S], F32R, tag="ao")  # normalized attn out ^T

        identity = const.tile([P, P], XDT, tag="ident")
        ones64 = const.tile([1, DH], F32R, tag="ones64")

        # wv + xv stream first (V projection overlaps its own DMA and the
        # q/k stream); q/k weights are emitted after the xv chunks below
        nc.sync.dma_start(out=wv_sb[:], in_=wv[:])

        from concourse.masks import make_identity
        if ATT_F16:
            make_identity(nc, identity[:])
            ones_dram = nc.inline_tensor(np.ones((P, KB), np.float16), name="ones_c")
            ones_ap = ones_dram.ap()
        else:
            make_identity(nc, identity[:].bitcast(F32))
            ones_dram = nc.inline_tensor(np.ones((P, KB), np.float32), name="ones_c")
            ones_ap = ones_dram.ap().bitcast(F32R)

        # --- Q/K projections (dc-outer over 8 psum accumulators each) ---
        def projection(src, wsb, dst, pproj, evac_dve=False):
            psums = [
                pproj.tile([P, 512], F32, tag="pp", name=f"pp{i}")
                for i in range(2 * NS)
            ]
            for dc in range(KD):
                xc = xpool.tile([P, S], XDT, tag="xc", name=f"xc{dc}")
                nc.sync.dma_start(out=xc[:], in_=src[:, dc, :])
                for jb in range(2):
                    for stl in range(NS):
                        nc.tensor.matmul(
                            psums[jb * NS + stl][:],
                            wsb[:, dc, jb * P:(jb + 1) * P],
                            xc[:, stl * 512:(stl + 1) * 512],
                            start=(dc == 0),
                            stop=(dc == KD - 1),
                        )
            # evacuation (jb0 first); ScalarE is idle before the first exp
            for jb in range(2):
                for stl in range(NS):
                    d = dst[:, jb, stl * 512:(stl + 1) * 512]
                    if evac_dve:
                        nc.vector.tensor_copy(d, psums[jb * NS + stl][:])
                    else:
                        nc.scalar.copy(d, psums[jb * NS + stl][:])

        with tc.tile_pool(name="pproj", bufs=8, space="PSUM") as pproj:
            projection(xv, wv_sb, vt_sb, pproj, evac_dve=True)
            nc.sync.dma_start(out=wq_sb[:], in_=wq[:])
            nc.sync.dma_start(out=wk_sb[:], in_=wk[:])
            projection(xq, wq_sb, qt_sb, pproj)
            projection(xk, wk_sb, kt_sb, pproj)
        nc.sync.dma_start(out=wo_sb[:], in_=wo[:])
        for h in range(HL):
            nc.sync.dma_start(out=v_sb[:, :, h * (DH + 1) + DH], in_=ones_ap)
        ones_f32 = nc.inline_tensor(np.ones((1, DH), np.float32), name="ones_f")
        nc.sync.dma_start(out=ones64[:], in_=ones_f32.ap().bitcast(F32R))

        # --- attention + deferred V pipeline + interleaved output proj ---
        with tc.tile_pool(name="psc", bufs=2, space="PSUM") as psc, tc.tile_pool(
            name="poacc", bufs=2, space="PSUM"
        ) as poacc, tc.tile_pool(name="expp", bufs=EX_BUFS) as expp, tc.tile_pool(
            name="npool", bufs=2
        ) as npool, tc.tile_pool(name="ostage", bufs=4) as opool:

            # V transposes, emitted lazily inside the first heads' kb-loops
            # (vt_sb is ready before attention starts; these fill PE slack
            # and borrow the spare "oacc" PSUM slot)
            def vjob_transpose(sb, jb):
                def f():
                    tp = poacc.tile([P, P], XDT, tag="oacc",
                                    name=f"tp_{sb}_{jb}")
                    nc.tensor.transpose(
                        tp[:, :P], vt_sb[:, jb, sb * P:(sb + 1) * P], identity[:]
                    )
                    for hh in range(2):
                        h = jb * 2 + hh
                        nc.vector.tensor_copy(
                            v_sb[:, sb, h * (DH + 1):h * (DH + 1) + DH],
                            tp[:, hh * DH:(hh + 1) * DH],
                        )
                return f

            vjobs = []
            for sb in range(KB):
                vjobs.append(vjob_transpose(sb, 0))
                vjobs.append(vjob_transpose(sb, 1))

            def kb_loop(qh, h, vjob_budget=0, mid_cb=None, norm_cb=None):
                q0 = qh * 1024
                jb = h // 2
                off = DH * (h % 2)
                oacc = poacc.tile([DH + 1, 1024], F32, tag="oacc")
                for kb in range(KB):
                    sc = psc.tile([P, 1024], F32, tag="sc")
                    for n in range(2):
                        nc.tensor.matmul(
                            sc[:, n * 512:(n + 1) * 512],
                            kt_sb[off:off + DH, jb, kb * P:(kb + 1) * P],
                            qt_sb[off:off + DH, jb, q0 + n * 512:q0 + (n + 1) * 512],
                            start=True,
                            stop=True,
                        )
                    ex = expp.tile([P, 1024], XDT, tag="ex")
                    nc.scalar.activation(ex[:], sc[:], AF.Exp)
                    for _ in range(vjob_budget):
                        if vjobs:
                            vjobs.pop(0)()
                    if norm_cb is not None and kb == KB // 4:
                        norm_cb()
                    if mid_cb is not None and kb == KB // 2:
                        mid_cb()
                    for n in range(2):
                        nc.tensor.matmul(
                            oacc[:, n * 512:(n + 1) * 512],
                            v_sb[:, kb, h * (DH + 1):(h + 1) * (DH + 1)],
                            ex[:, n * 512:(n + 1) * 512],
                            start=(kb == 0),
                            stop=(kb == KB - 1),
                        )
                recip = npool.tile([1, 1024], F32R, tag="recip")
                with nc.allow_low_precision(reason="fp32r softmax denom"):
                    nc.vector.reciprocal(recip[:], oacc[DH:DH + 1, :])
                return oacc, recip

            def normalize(task_state):
                (qh, h), (oacc, recip) = task_state
                q0 = qh * 1024
                jb = h // 2
                off = DH * (h % 2)
                bc = psc.tile([DH, 1024], F32, tag="sc")
                for n in range(2):
                    nc.tensor.matmul(
                        bc[:, n * 512:(n + 1) * 512],
                        ones64[:],
                        recip[:, n * 512:(n + 1) * 512],
                        start=True,
                        stop=True,
                    )
                bcast = npool.tile([DH, 1024], F32, tag="bcast")
                nc.vector.tensor_copy(bcast[:], bc[:])
                nc.vector.tensor_mul(
                    ao_sb[off:off + DH, jb, q0:q0 + 1024],
                    oacc[0:DH, :],
                    bcast[:],
                )

            def oproj_slice(qh, ebs):
                q0 = qh * 1024
                for eb in ebs:
                    ob = opool.tile([P, 1024], F32, tag="ob")
                    for stl in range(2):
                        s0 = q0 + stl * 512
                        po = poacc.tile([P, 512], F32, tag="oacc",
                                        name=f"po_{qh}_{eb}_{stl}")
                        for jb in range(2):
                            nc.tensor.matmul(
                                po[:, :512],
                                wo_sb[:, jb, eb * P:(eb + 1) * P],
                                ao_sb[:, jb, s0:s0 + 512],
                                start=(jb == 0),
                                stop=(jb == 1),
                            )
                        d = ob[:, stl * 512:(stl + 1) * 512]
                        if qh == 1 and stl == 0:
                            nc.scalar.copy(d, po[:, :512])  # ACT idle at tail
                        else:
                            nc.vector.tensor_copy(d, po[:, :512])
                    nc.sync.dma_start(out=out_t[eb][:, q0:q0 + 1024], in_=ob[:])

            tasks = [(qh, h) for qh in range(QH) for h in range(HL)]
            pending = [None]
            for i, (qh, h) in enumerate(tasks):
                # sprinkle V transposes into the first task's PE slack;
                # the previous task's normalize lands at kb=4 (frees its
                # accumulator slot); qh0's output projection creeps through
                # qh1 one eb at a time (kb=8 + task end)
                def norm_prev():
                    if pending[0] is not None:
                        normalize(pending[0])
                        pending[0] = None
                mid = None
                if 4 <= i <= 7:
                    eb_mid = (i - 4) * 2
                    mid = lambda e=eb_mid: oproj_slice(0, [e])
                state = kb_loop(qh, h, vjob_budget=4 if i < 1 else 0,
                                mid_cb=mid, norm_cb=norm_prev)
                assert not vjobs or i < 1
                pending[0] = ((qh, h), state)
                if 4 <= i <= 7:
                    oproj_slice(0, [(i - 4) * 2 + 1])
            normalize(pending[0][1] and pending[0])
            oproj_slice(1, range(EB))

    nc.finalize()
    return nc


_NC_CACHE = None


def _get_nc():
    global _NC_CACHE
    if _NC_CACHE is None:
        _NC_CACHE = build_nc()
    return _NC_CACHE


def make_in_maps(query, key, value, Wq, Wk, Wv, Wo):
    """Build the 8 per-core input dicts from the full tensors (p-major)."""
    query = np.asarray(query, np.float32)
    key = np.asarray(key, np.float32)
    value = np.asarray(value, np.float32)
    Wq = np.asarray(Wq, np.float32)
    Wk = np.asarray(Wk, np.float32)
    Wv = np.asarray(Wv, np.float32)
    Wo = np.asarray(Wo, np.float32)
    xdt = np.float16 if ATT_F16 else np.float32

    def pmajor(a2d, inner):  # [Drows, inner] -> [P, Drows//P, inner]
        return np.ascontiguousarray(
            a2d.reshape(KD, P, inner).transpose(1, 0, 2)
        )

    scale = np.float32(1.0 / np.sqrt(DH))
    xs = {}
    for b in range(B):
        xs[b] = {
            "xq": pmajor(np.ascontiguousarray(query[b].T), S).astype(xdt),
            "xk": pmajor(np.ascontiguousarray(key[b].T), S).astype(xdt),
            "xv": pmajor(np.ascontiguousarray(value[b].T), S).astype(xdt),
        }
    ws = {}
    for hg in range(4):
        sl = slice(hg * J, (hg + 1) * J)
        wo_t = np.ascontiguousarray(Wo[:, sl].T)  # [256, 1024]
        ws[hg] = {
            "wq": pmajor(np.ascontiguousarray(Wq[sl].T * scale), J).astype(xdt),
            "wk": pmajor(np.ascontiguousarray(Wk[sl].T), J).astype(xdt),
            "wv": pmajor(np.ascontiguousarray(Wv[sl].T), J).astype(xdt),
            "wo": np.ascontiguousarray(
                wo_t.reshape(2, P, D).transpose(1, 0, 2)
            ),
        }
    in_maps = []
    for c in range(NCORES):
        b, hg = c // 4, c % 4
        m = {}
        m.update(xs[b])
        m.update(ws[hg])
        in_maps.append(m)
    return in_maps


def assemble(results, bo):
    """Sum the 4 per-core partials per batch, add bo."""
    bo = np.asarray(bo, np.float32)
    out = np.zeros((B, S, D), np.float32)
    for c in range(NCORES):
        b = c // 4
        part = results[c]["out_t"].reshape(D, S).T  # [S, D]
        out[b] += part
    out += bo[None, None, :]
    return out


def kernel(query, key, value, Wq, Wk, Wv, Wo, bo):
    import os
    import time

    # helps recover wedged NeuronCores between runs
    os.environ.setdefault("NEURON_RT_RESET_CORES", "1")
    from concourse.bass_utils import run_bass_kernel_spmd

    nc = _get_nc()
    in_maps = make_in_maps(query, key, value, Wq, Wk, Wv, Wo)
    last_exc = None
    for attempt in range(3):
        try:
            res = run_bass_kernel_spmd(nc, in_maps, list(range(NCORES)))
            return assemble(res.results, bo)
        except Exception as e:  # transient NRT_EXEC_UNIT_UNRECOVERABLE etc.
            last_exc = e
            time.sleep(2.0)
    raise last_exc



# revision 41
# speedup vs baseline: 1.2641x; 1.2641x over previous
"""Multi-head attention (B=2, S=2048, D=1024, H=16, Dh=64) on 8 TRN2 cores.

Sharding: data-parallel over batch (2) x tensor-parallel over heads (16 -> 4
groups of 4). Core c handles batch c//4, heads [4*(c%4), 4*(c%4)+4).
Each core computes its partial output projection (Wo column slice); the host
sums the 4 partials per batch (the "all-reduce") and adds bo.

v2 redesign (cost-model driven; ACT exp is the ~133us floor, PE ~140us):
  - attn@V is SWAPPED: stationary = ex-tile [128 kpos, 128 q], moving =
    [V|ones] f16 [128, 65] -> psum [128 q, 65] (denominator in col 64).
    Halves the attn@V PE cost vs the [65, q] orientation (cost = out free
    size), and turns the softmax denominator into a per-partition scalar
    (cheap DVE normalize, no ones-broadcast matmul).
  - V is projected s-major directly (stationary = x chunk, moving = Wv) so
    no V transposes are needed.
  - ao comes out q-major; PE-transposes (8 per task, [128,64]->[64,128] f16)
    restore j-major for the output projection. Odd heads transpose straight
    into psum partitions 64-127 (tile_position), so evacuation copies are
    partition-aligned.
  - all matmuls f16 (fp8 would break the 2e-2 gate: logit noise ~ final rel
    err, no averaging), out_t f16, wo f16.
  - accs accumulate with start=False onto DVE-memset-zeroed psum so the
    multi-accumulator-per-bank layout survives HW pending-zero semantics.
  - PSUM banks: scores 2x[128,1024]=4 (transposes borrow this arena's slots),
    acc 2x[128,4,65] bufs=1 = 2, V-proj psum 1, o-proj psum 1 -> 8 exactly.
  - slot scheduler: scores+exp are the ACT-paced skeleton; projections,
    V-jobs, accs (lagged), normalizes, transposes and o-proj slices fill PE
    slack with ready/deadline bookkeeping. acc lag decays 1/task (acc bufs=1
    forces monotone decay); o-proj for the second q-half runs post-ladder
    with a wide psum pool.
"""

import numpy as np
from contextlib import ExitStack

import concourse.bass as bass
from concourse import bacc
import concourse.mybir as mybir
import concourse.tile as tile

F32 = mybir.dt.float32
F16 = mybir.dt.float16
AF = mybir.ActivationFunctionType

B = 2
S = 2048
D = 1024
H = 16
DH = 64
NCORES = 8
HL = 4          # heads per core
J = HL * DH     # 256 local projection width
P = 128
KD = D // P     # 8 d-chunks
KB = S // P     # 16 k-blocks of 128
QH = 2          # q-halves of 1024
EB = D // P     # 8 e-blocks
SQ = 4          # s-quarters of 512 (projection granule)
VW = DH + 1     # V + ones column


def build_nc():
    nc = bacc.Bacc()

    xq = nc.dram_tensor("xq", [P, KD, S], F16, kind="ExternalInput")
    xk = nc.dram_tensor("xk", [P, KD, S], F16, kind="ExternalInput")
    xv = nc.dram_tensor("xv", [P, KD, S], F16, kind="ExternalInput")
    wq = nc.dram_tensor("wq", [P, KD, J], F16, kind="ExternalInput")
    wk = nc.dram_tensor("wk", [P, KD, J], F16, kind="ExternalInput")
    wv = nc.dram_tensor("wv", [P, KD, J], F16, kind="ExternalInput")
    wo = nc.dram_tensor("wo", [P, 2, D], F16, kind="ExternalInput")
    # per-jb partial output projections; the host sums the two halves
    # (it already sums the 4 cores' partials)
    out_t = nc.dram_tensor("out_t", [2, EB, P, S], F16, kind="ExternalOutput")

    with tile.TileContext(nc) as tc, ExitStack() as st:
        const = st.enter_context(tc.tile_pool(name="const", bufs=1))
        persist = st.enter_context(tc.tile_pool(name="persist", bufs=1))

        wq_sb = const.tile([P, KD, J], F16, tag="wq")
        wk_sb = const.tile([P, KD, J], F16, tag="wk")
        wv_sb = const.tile([P, KD, J], F16, tag="wv")
        wo_sb = const.tile([P, 2, D], F16, tag="wo")
        identity = const.tile([P, P], F16, tag="ident")

        xq_sb = persist.tile([P, KD, S], F16, tag="xq")
        xk_sb = persist.tile([P, KD, S], F16, tag="xk")
        xv_sb = persist.tile([P, KD, S], F16, tag="xv")
        qt_sb = persist.tile([P, 2, S], F16, tag="qt")   # Q_T [256, 2048]
        kt_sb = persist.tile([P, 2, S], F16, tag="kt")   # K_T
        v_sb = persist.tile([P, KB, HL, VW], F16, tag="v")  # V s-major + ones
        ao_sb = persist.tile([P, 2, S], F16, tag="ao")   # normalized attn ^T

        from concourse.masks import make_identity
        make_identity(nc, identity[:])
        ones_dram = nc.inline_tensor(np.ones((P, KB), np.float16), name="ones_c")

        # ---- all input DMAs up front; queue order = priority order ----
        def dma_x(dst, src, q0, q1):
            nc.sync.dma_start(out=dst[:, :, 512 * q0:512 * q1],
                              in_=src[:, :, 512 * q0:512 * q1])

        nc.sync.dma_start(out=wk_sb[:], in_=wk[:])
        dma_x(xk_sb, xk, 0, 1)
        nc.sync.dma_start(out=wq_sb[:], in_=wq[:])
        dma_x(xq_sb, xq, 0, 1)
        dma_x(xq_sb, xq, 1, 2)
        dma_x(xk_sb, xk, 1, 2)
        dma_x(xk_sb, xk, 2, 3)
        nc.sync.dma_start(out=wv_sb[:], in_=wv[:])
        dma_x(xv_sb, xv, 0, 1)
        dma_x(xv_sb, xv, 1, 2)
        dma_x(xk_sb, xk, 3, 4)
        dma_x(xv_sb, xv, 2, 3)
        dma_x(xv_sb, xv, 3, 4)
        dma_x(xq_sb, xq, 2, 3)
        dma_x(xq_sb, xq, 3, 4)
        nc.sync.dma_start(out=wo_sb[:], in_=wo[:])
        for h in range(HL):
            nc.sync.dma_start(out=v_sb[:, :, h, DH], in_=ones_dram.ap())

        # ---------------- job bodies ----------------
        # paux: ONE shared 2-bank psum arena ([128,512] f32 slots, tag "pp")
        # used in turn by Q/K projection quarters, V-projection jobs, and the
        # in-ladder qh0 o-proj slices (temporally interleaved; rotation WAR
        # deps keep it safe).
        paux = st.enter_context(
            tc.tile_pool(name="paux", bufs=2, space="PSUM"))

        def qk_proj(x_sb, w_sb, dst, jb, sq, evac_act, width=512):
            """One s-chunk of a Q/K projection column-block: 8 dc matmuls
            accumulating [128, width], evacuated to qt/kt f16."""
            ps = paux.tile([P, 512], F32, tag="pp",
                           name=f"pp{jb}_{sq}_{width}")[:, 0:width]
            for dc in range(KD):
                nc.tensor.matmul(
                    ps,
                    w_sb[:, dc, jb * P:(jb + 1) * P],
                    x_sb[:, dc, width * sq:width * (sq + 1)],
                    start=(dc == 0),
                    stop=(dc == KD - 1),
                )
            d = dst[:, jb, width * sq:width * (sq + 1)]
            if evac_act:
                nc.scalar.copy(d, ps)
            else:
                nc.vector.tensor_copy(d, ps)

        def v_proj(kb):
            """V s-major: stationary xv s-slice, moving wv -> [128 s, 256 j],
            evac strided into v_sb (skipping the ones columns)."""
            ps = paux.tile([P, 512], F32, tag="pp", name=f"pv{kb}")[:, 0:256]
            for dc in range(KD):
                nc.tensor.matmul(
                    ps,
                    xv_sb[:, dc, kb * P:(kb + 1) * P],
                    wv_sb[:, dc, :],
                    start=(dc == 0),
                    stop=(dc == KD - 1),
                )
            src = ps.rearrange("p (h d) -> p h d", h=HL)
            nc.vector.tensor_copy(v_sb[:, kb, :, 0:DH], src)

        # ---------------- attention-phase pools ----------------
        psc = st.enter_context(tc.tile_pool(name="psc", bufs=2, space="PSUM"))
        pacc = st.enter_context(tc.tile_pool(name="pacc", bufs=1, space="PSUM"))
        expp = st.enter_context(tc.tile_pool(name="expp", bufs=15))
        aoq = st.enter_context(tc.tile_pool(name="aoq", bufs=2))
        rpool = st.enter_context(tc.tile_pool(name="rpool", bufs=2))
        opool = st.enter_context(tc.tile_pool(name="ostage", bufs=4))

        NT = QH * HL
        # ladder order (h, qh): jb1 heads (h2/h3) come last so the K/Q jb1
        # projections aren't needed until slot 64 — spreads the filler load
        TASKS = [(0, 0), (1, 0), (0, 1), (1, 1),
                 (2, 0), (3, 0), (2, 1), (3, 1)]
        ex_tiles = {}        # (t, kb) -> ex tile
        acc_tiles = {}       # t -> (accA, accB)
        aoq_tiles = {}       # t -> ao_q tile

        def task_qh(t):
            return TASKS[t][1]

        def task_h(t):
            return TASKS[t][0]

        def scores_exp(t, kb):
            qh, h = task_qh(t), task_h(t)
            q0 = qh * 1024
            jb = h // 2
            off = DH * (h % 2)
            sc = psc.tile([P, 1024], F32, tag="sc")
            for n in range(2):
                nc.tensor.matmul(
                    sc[:, n * 512:(n + 1) * 512],
                    kt_sb[off:off + DH, jb, kb * P:(kb + 1) * P],
                    qt_sb[off:off + DH, jb, q0 + n * 512:q0 + (n + 1) * 512],
                    start=True,
                    stop=True,
                )
            ex = expp.tile([P, 1024], F16, tag="ex", name=f"ex{t}_{kb}")
            nc.scalar.activation(ex[:], sc[:], AF.Exp)
            ex_tiles[(t, kb)] = ex

        def acc_group(t, kb):
            """8 swapped attn@V matmuls for (t, kb): stationary ex q-block,
            moving [V|1] -> acc[:, qb, 0:65] (start=False onto zeroed psum)."""
            h = task_h(t)
            if t not in acc_tiles:
                a = pacc.tile([P, 4, VW], F32, tag="accA", name=f"accA{t}")
                b = pacc.tile([P, 4, VW], F32, tag="accB", name=f"accB{t}")
                nc.vector.memset(a[:], 0.0)
                nc.vector.memset(b[:], 0.0)
                acc_tiles[t] = (a, b)
            a, b = acc_tiles[t]
            ex = ex_tiles.pop((t, kb))
            mv = v_sb[:, kb, h, :]
            for half in range(2):
                acc = (a, b)[half]
                for qb in range(4):
                    q = (half * 4 + qb) * P
                    nc.tensor.matmul(
                        acc[:, qb, :],
                        ex[:, q:q + P],
                        mv,
                        start=False,
                        stop=(kb == KB - 1),
                        skip_group_check=True,
                    )

        def normalize(t):
            a, b = acc_tiles.pop(t)
            recip = rpool.tile([P, 8], F32, tag="recip", name=f"rc{t}")
            with nc.allow_low_precision(reason="softmax denom reciprocal"):
                nc.vector.reciprocal(recip[:, 0:4], a[:, :, DH:VW])
                nc.vector.reciprocal(recip[:, 4:8], b[:, :, DH:VW])
            ao_q = aoq.tile([P, 8, DH], F16, tag="aoq", name=f"aoq{t}")
            for half in range(2):
                acc = (a, b)[half]
                rb = recip[:, half * 4:(half + 1) * 4].unsqueeze(2)
                rb = rb.broadcast_to([P, 4, DH])
                nc.vector.tensor_mul(
                    ao_q[:, half * 4:(half + 1) * 4, :],
                    acc[:, :, 0:DH],
                    rb,
                )
            aoq_tiles[t] = ao_q

        def transp(t):
            """8 PE transposes ao_q [128 q,64] -> [64,128] f16 into a borrowed
            scores-arena bank; odd heads land on psum partitions 64-127."""
            qh, h = task_qh(t), task_h(t)
            q0 = qh * 1024
            jb = h // 2
            base = DH * (h % 2)
            ao_q = aoq_tiles.pop(t)
            tp = psc.tile([P, 1024], F32, tag="sc", name=f"tp{t}")
            tpf = tp[:].bitcast(F16)  # [128, 2048] f16 view
            for qb in range(8):
                nc.tensor.transpose(
                    tpf[base:base + DH, qb * P:(qb + 1) * P],
                    ao_q[:, qb, :],
                    identity[:],
                )
            nc.vector.tensor_copy(
                ao_sb[base:base + DH, jb, q0:q0 + 1024],
                tpf[base:base + DH, 0:1024],
            )

        def _evac(eng, d, ps):
            # GPSIMD cannot access PSUM; evacs go to DVE (or ACT when idle)
            if eng == "act":
                nc.scalar.copy(d, ps)
            else:
                nc.vector.tensor_copy(d, ps)

        def oproj_part(qh, jb, eb, pool, evac_eng):
            """One (q-half, jb-half) partial o-proj slice: single-matmul
            psum per 512 cols, evac f16, DMA; the HOST sums the jb halves."""
            q0 = qh * 1024
            ob = opool.tile([P, 1024], F16, tag="ob", name=f"ob{qh}{jb}{eb}")
            for stl in range(2):
                s0 = q0 + stl * 512
                ps = pool.tile([P, 512], F32, tag="pp",
                               name=f"po{qh}{jb}{eb}{stl}")
                nc.tensor.matmul(
                    ps[:],
                    wo_sb[:, jb, eb * P:(eb + 1) * P],
                    ao_sb[:, jb, s0:s0 + 512],
                    start=True,
                    stop=True,
                )
                _evac(evac_eng, ob[:, stl * 512:(stl + 1) * 512], ps[:])
            nc.sync.dma_start(out=out_t[jb, eb][:, q0:q0 + 1024], in_=ob[:])

        # ---------------- pre-ladder ----------------
        qk_proj(xk_sb, wk_sb, kt_sb, 0, 0, True)
        qk_proj(xq_sb, wq_sb, qt_sb, 0, 0, True)
        qk_proj(xq_sb, wq_sb, qt_sb, 0, 1, True)

        # ---------------- filler job list ----------------
        # (ready_slot, deadline_slot, cycles, fn); deadline None = soft
        jobs = []

        def add_job(ready, deadline, cy, fn):
            jobs.append([ready, deadline if deadline is not None else 10**9,
                         cy, fn])

        # All remaining projections as s-EIGHTH jobs (2048 cy) so the token
        # bucket can spread them smoothly. Deadlines: kt-jb0 eighth e feeds
        # t0 kb-pair at slot 2e; kt-jb1 at t2 (32+2e); qt halves at task
        # starts. Ready slots track the serial DMA schedule.
        def pj(x_sb, w_sb, dst, jb, e):
            return lambda: qk_proj(x_sb, w_sb, dst, jb, e, False, width=256)

        for e in range(2, 8):  # K jb0 eighths 2-7 (0-1 done pre-ladder)
            add_job(0, 2 * e - 1, 2048, pj(xk_sb, wk_sb, kt_sb, 0, e))
        for e in range(4, 8):  # Q jb0 s 1024:2048 (t2 = (h0, qh1), slot 32)
            add_job(21 + (e - 4), 29 + (e - 4) // 2, 2048,
                    pj(xq_sb, wq_sb, qt_sb, 0, e))
        for e in range(8):     # K jb1 (t4 = (h2, qh0), slot 64)
            add_job(1 + (e // 2) * 2, 63 + 2 * e, 2048,
                    pj(xk_sb, wk_sb, kt_sb, 1, e))
        for e in range(4):     # Q jb1 s 0:1024 (t4)
            add_job(0, 62, 2048, pj(xq_sb, wq_sb, qt_sb, 1, e))
        for e in range(4, 8):  # Q jb1 s 1024:2048 (t6, slot 96)
            add_job(24 + (e - 4), 93 + (e - 4) // 2, 2048,
                    pj(xq_sb, wq_sb, qt_sb, 1, e))
        # V jobs: xv quarter kb//4 lands ~slot 10+3*(kb//4)
        # LAG must decay by exactly 1/task: norm[t] shares the slot of
        # acc[t][15] and must precede acc[t+1][0] in emission order (acc
        # bufs=1: the next task's memset may only be emitted after the
        # previous normalize has been).
        LAG = [12, 11, 10, 9, 8, 7, 6, 5]
        for kb in range(KB):
            add_job(11 + 3 * (kb // 4), LAG[0] + kb - 1, 2048,
                    (lambda k: lambda: v_proj(k))(kb))
        # transp[t] emission slot (o-proj emission must come after the
        # transposes whose ao_sb bytes it reads, else Tile records no dep).
        # o-proj part (qh, jb) needs the transposes of heads 2jb and 2jb+1
        # at that q-half.
        TR = [16 * t + LAG[t] + KB + 1 for t in range(NT)]

        def part_ready(qh, jb):
            return max(TR[TASKS.index((2 * jb, qh))],
                       TR[TASKS.index((2 * jb + 1, qh))]) + 1

        for pi, (pqh, pjb) in enumerate([(0, 0), (1, 0), (0, 1)]):
            r0 = part_ready(pqh, pjb)
            for i, eb in enumerate(range(EB)):
                add_job(r0 + 3 * i, None, 1024,
                        (lambda q, j, e: lambda: oproj_part(
                            q, j, e, paux, "dve"))(pqh, pjb, eb))

        # acc/norm/transp schedule keyed by slot (insertion order within a
        # slot follows task order, which keeps norm[t] before acc[t+1][0])
        slot_actions = {}

        def at_slot(n, fn):
            slot_actions.setdefault(n, []).append(fn)

        for t in range(NT):
            for kb in range(KB):
                at_slot(16 * t + LAG[t] + kb,
                        (lambda tt, kk: lambda: acc_group(tt, kk))(t, kb))
            at_slot(16 * t + LAG[t] + KB - 1,
                    (lambda tt: lambda: normalize(tt))(t))
            at_slot(TR[t],
                    (lambda tt: lambda: transp(tt))(t))

        # ---------------- the ladder ----------------
        # Token-bucket filler budget: sustainable filler rate is ACT-pace
        # (2491 cy/slot) minus the scores+acc skeleton (~1550 cy) ~= 900;
        # credit carries across slots (capped) so dry spells don't turn
        # into later bursts that starve ACT.
        SLOT_BUDGET = 950
        CREDIT_CAP = 2600
        credit = [0]

        def run_slot(n):
            for fn in slot_actions.pop(n, []):
                fn()
            # forced (deadline) jobs run regardless and consume credit
            for j in sorted([j for j in jobs if j[1] <= n + 1],
                            key=lambda j: j[1]):
                jobs.remove(j)
                j[3]()
                credit[0] -= j[2]
            credit[0] = min(credit[0] + SLOT_BUDGET, CREDIT_CAP)
            while credit[0] > 0:
                ready = [j for j in jobs if j[0] <= n]
                if not ready:
                    break
                j = min(ready, key=lambda j: j[1])
                jobs.remove(j)
                j[3]()
                credit[0] -= j[2]

        for t in range(NT):
            for kb in range(KB):
                n = 16 * t + kb
                scores_exp(t, kb)
                run_slot(n)

        # ---------------- tail ----------------
        n = 16 * NT
        while slot_actions or jobs:
            run_slot(n)
            n += 1
            if n > 16 * NT + 64:
                for fn in [f for acts in slot_actions.values() for f in acts]:
                    fn()
                slot_actions.clear()
                for j in list(jobs):
                    j[3]()
                jobs.clear()

        # tail: the (qh1, jb1) o-proj parts (need transp(t7)); ACT is idle
        # post-ladder so it handles the evacs.
        for eb in range(EB):
            oproj_part(1, 1, eb, paux, "act")

    nc.finalize()
    return nc


_NC_CACHE = None


def _get_nc():
    global _NC_CACHE
    if _NC_CACHE is None:
        _NC_CACHE = build_nc()
    return _NC_CACHE


def make_in_maps(query, key, value, Wq, Wk, Wv, Wo):
    """Build the 8 per-core input dicts from the full tensors (p-major)."""
    query = np.asarray(query, np.float32)
    key = np.asarray(key, np.float32)
    value = np.asarray(value, np.float32)
    Wq = np.asarray(Wq, np.float32)
    Wk = np.asarray(Wk, np.float32)
    Wv = np.asarray(Wv, np.float32)
    Wo = np.asarray(Wo, np.float32)

    def pmajor(a2d, inner):  # [Drows, inner] -> [P, Drows//P, inner]
        return np.ascontiguousarray(
            a2d.reshape(KD, P, inner).transpose(1, 0, 2)
        )

    scale = np.float32(1.0 / np.sqrt(DH))
    xs = {}
    for b in range(B):
        xs[b] = {
            "xq": pmajor(np.ascontiguousarray(query[b].T), S).astype(np.float16),
            "xk": pmajor(np.ascontiguousarray(key[b].T), S).astype(np.float16),
            "xv": pmajor(np.ascontiguousarray(value[b].T), S).astype(np.float16),
        }
    ws = {}
    for hg in range(4):
        sl = slice(hg * J, (hg + 1) * J)
        wo_t = np.ascontiguousarray(Wo[:, sl].T)  # [256, 1024]
        ws[hg] = {
            "wq": pmajor(np.ascontiguousarray(Wq[sl].T * scale), J).astype(np.float16),
            "wk": pmajor(np.ascontiguousarray(Wk[sl].T), J).astype(np.float16),
            "wv": pmajor(np.ascontiguousarray(Wv[sl].T), J).astype(np.float16),
            "wo": np.ascontiguousarray(
                wo_t.reshape(2, P, D).transpose(1, 0, 2)
            ).astype(np.float16),
        }
    in_maps = []
    for c in range(NCORES):
        b, hg = c // 4, c % 4
        m = {}
        m.update(xs[b])
        m.update(ws[hg])
        in_maps.append(m)
    return in_maps


def assemble(results, bo):
    """Sum the 4 per-core partials per batch, add bo."""
    bo = np.asarray(bo, np.float32)
    out = np.zeros((B, S, D), np.float32)
    for c in range(NCORES):
        b = c // 4
        part = results[c]["out_t"].astype(np.float32).sum(axis=0).reshape(D, S).T
        out[b] += part
    out += bo[None, None, :]
    return out


def kernel(query, key, value, Wq, Wk, Wv, Wo, bo):
    import os
    import time

    # helps recover wedged NeuronCores between runs
    os.environ.setdefault("NEURON_RT_RESET_CORES", "1")
    from concourse.bass_utils import run_bass_kernel_spmd

    nc = _get_nc()
    in_maps = make_in_maps(query, key, value, Wq, Wk, Wv, Wo)
    last_exc = None
    for attempt in range(3):
        try:
            res = run_bass_kernel_spmd(nc, in_maps, list(range(NCORES)))
            return assemble(res.results, bo)
        except Exception as e:  # transient NRT_EXEC_UNIT_UNRECOVERABLE etc.
            last_exc = e
            time.sleep(2.0)
    raise last_exc
